# revision 26
# baseline (speedup 1.0000x reference)
"""CQAttention (BiDAF-style context-query attention) on 8 TRN2 NeuronCores.

Full shapes: contex [64, 512, 256], question [64, 64, 256],
W_weight [1, 768], W_bias [1] -> out [64, 512, 1024].

Sharding: pure data-parallel over batch, 8 batches per core.

Math notes (per batch, C=[512,256], Q=[64,256], w=[wq|wc|wi]):
  S[i,j] = sum_d C[i,d]*wi[d]*Q[j,d] + C[i].wc + Q[j].wq + b
  S1 = softmax_j(S), S2 = softmax_i(S)
  - b drops out of both softmaxes; s_c drops out of S1; s_q drops out of S2.
  - E1 = exp(s_i + s_q[j]), r1[i] = sum_j E1;  S1 = E1/r1
  - E2 = exp(s_i + s_c[i]), r2[j] = sum_i E2;  S2 = E2/r2
  - A  = S1 @ Q = (E1 @ Q)/r1
  - Bm = (S1 @ S2^T) @ C = S1 @ (S2^T @ C) = (E1 @ C2)/r1, C2 = (E2^T @ C)/r2
  r1/r2 are obtained for free as ones-columns appended to the matmul rhs.
  out = [C | A | C*A | C*Bm]

DMA design:
  - context rows are mapped i = 4p + t (partition-major): C loads move
    4KB-contiguous lines; the merged [A|C*A|C*Bm] store moves 3KB lines.
  - ALL input DMAs are issued up front (before any compute is emitted) into
    persistent tiles, so no load ever queues behind compute on its issuing
    engine.  C batch 0 rides the sync ring in parallel with Q on the
    scalar ring so batch 0 can start ASAP.
  - The C output block is stored straight from the persistent C_all input
    tile on the scalar ring (idle after the loads drain) — no copy.
  - The other three blocks are assembled in one [128, 4, 768] staging tile
    and shipped as a single 1.5MB store on the sync ring.

Emission is a 4-stage software pipeline; each "step" emits, in this order,
  S4(b-3): M2/M4 + normalization/products + store   (uses E1,C2 from b-3)
  S3(b-2): M3 + 1/r2 + C2
  S2(b-1): M1T/M1' + exps
  S1(b):   casts, Q'*wi, s_q, PE transposes of C
Reverse-stage order puts instructions whose inputs are oldest (most likely
ready) at the head of every engine queue, which keeps the in-order engines
from head-of-line blocking on same-step dependency chains.
"""

import numpy as np

B, LC, LQ, D = 64, 512, 64, 256
NCORES = 8
BL = B // NCORES  # batches per core
NSLOT = 5

_NC_CACHE = None


def _build_nc():
    import concourse.bass as bass
    import concourse.mybir as mybir
    from concourse import bacc
    from concourse import masks
    from concourse import tile
    from contextlib import ExitStack

    f32 = mybir.dt.float32
    bf16 = mybir.dt.bfloat16
    AF = mybir.ActivationFunctionType
    MUL = mybir.AluOpType.mult
    ts = bass.ts

    nc = bacc.Bacc("TRN2", target_bir_lowering=False, debug=False)
    C_d = nc.dram_tensor("contex", [BL, LC, D], f32, kind="ExternalInput")
    Q_d = nc.dram_tensor("question", [BL, LQ, D], f32, kind="ExternalInput")
    W_d = nc.dram_tensor("W_weight", [1, 3 * D], f32, kind="ExternalInput")
    out_d = nc.dram_tensor("out", [BL, LC, 4 * D], f32, kind="ExternalOutput")

    with tile.TileContext(nc) as tc, ExitStack() as ctx:
        const = ctx.enter_context(tc.tile_pool(name="const", bufs=1))
        sb = ctx.enter_context(tc.tile_pool(name="sb", bufs=NSLOT))
        stg = ctx.enter_context(tc.tile_pool(name="stg", bufs=3))
        ps_tc = ctx.enter_context(tc.tile_pool(name="ps_tc", bufs=2, space="PSUM"))
        ps_si = ctx.enter_context(tc.tile_pool(name="ps_si", bufs=2, space="PSUM"))
        ps_mm = ctx.enter_context(tc.tile_pool(name="ps_mm", bufs=4, space="PSUM"))

        # ---- all input DMAs, issued before any compute exists ----
        # sync ring: C batch 0 FIRST (it gates all of batch 0's compute),
        # then the two small weight views (small DMAs cost ~1.5us each on
        # the ring, so they go after C0 and are merged where possible)
        C_all = const.tile([128, BL, 4, D], f32, tag="C_all")
        nc.sync.dma_start(C_all[:, 0], C_d[0].rearrange("(p t) d -> p t d", t=4))
        W3 = const.tile([1, 3, D], f32, tag="W3")
        nc.sync.dma_start(W3[:], W_d.rearrange("o (k d) -> o k d", d=D))
        wc_f32 = const.tile([128, 2, 1], f32, tag="wc_f32")
        nc.sync.dma_start(
            wc_f32[:], W_d[0, D : 2 * D].rearrange("(k p o) -> p k o", p=128, o=1)
        )

        # scalar ring: Q, then the remaining C batches (4KB lines)
        Q_all = const.tile([LQ, BL, D], f32, tag="Q_all")
        nc.scalar.dma_start(Q_all[:], Q_d.rearrange("b j d -> j b d"))
        for b in range(1, BL):
            nc.scalar.dma_start(
                C_all[:, b], C_d[b].rearrange("(p t) d -> p t d", t=4)
            )

        # ---- constants ----
        ident = const.tile([128, 128], bf16, tag="ident")
        masks.make_identity(nc, ident[:])

        # persistent slotted bf16 C tiles: the ones columns are written once,
        # casts only rewrite cols 0:256 each time a slot is reused
        C_bfs = const.tile([128, NSLOT, 4, D + 1], bf16, tag="C_bfs")
        nc.gpsimd.memset(C_bfs[:, :, :, D : D + 1], 1.0)

        W_b = const.tile([LQ, 3, D], f32, tag="W_b")
        QP_all = const.tile([LQ, BL, D], bf16, tag="QP_all")
        Q_bfs = const.tile([LQ, BL, D + 1], bf16, tag="Q_bfs")
        QW_all = const.tile([128, NSLOT, 2, 65], bf16, tag="QW_all")

        def w_chain():
            # broadcast wq/wc/wi rows to 64 partitions on gpsimd (no PE/ACT)
            nc.gpsimd.partition_broadcast(W_b[:], W3[:])
            # Q is small: precompute bf16 casts and Q' = Q*wi for ALL batches
            # in one shot, on gpsimd, off the DVE/ACT critical path
            nc.gpsimd.memset(Q_bfs[:, :, D : D + 1], 1.0)
            nc.gpsimd.tensor_copy(Q_bfs[:, :, 0:D], Q_all[:])
            qp_a, qp_b = bass.broadcast_tensor_aps(Q_all[:], W_b[:, 2:3, :])
            nc.gpsimd.tensor_mul(QP_all[:], qp_a, qp_b)
            # persistent slotted QW tiles: the wc columns are written once
            for s in range(NSLOT):
                nc.vector.tensor_copy(QW_all[:, s, :, 64:65], wc_f32[:])

        wq_b = W_b[:, 0, :]  # [64, 256] rows = wq

        s1c_out, st1, st2, st3 = {}, {}, {}, {}  # stage state, keyed by batch

        def s1c(b):
            s = b % NSLOT
            Cb = C_all[:, b]  # [128, 4, 256] f32
            C_bf = C_bfs[:, s]  # [128, 4, 257] bf16

            # ship output block 0 = C straight from the input tile
            # (scalar ring; it drains after the input loads finish)
            nc.scalar.dma_start(
                out_d[b].rearrange("(p t) dd -> p t dd", t=4)[:, :, 0:D], Cb
            )

            # C_bf cast in halves on two engines
            nc.vector.tensor_copy(C_bf[:, 0:2, 0:D], Cb[:, 0:2, :])
            nc.scalar.copy(C_bf[:, 2:4, 0:D], Cb[:, 2:4, :])

            # tc: C^T -> CT [128, 2, 512] (k = d-tile, free position t*128+p
            # corresponds to row i = 4p + t; consistent everywhere below).
            # Emitted before tq so the PE never waits on Q-side data first.
            tcp = ps_tc.tile([128, 2, 512], bf16, tag="tcp")
            for t in range(4):
                for k in range(2):
                    nc.tensor.transpose(
                        tcp[:, k, ts(t, 128)], C_bf[:, t, ts(k, 128)], ident[:]
                    )
            CT = sb.tile([128, 2, 512], bf16, tag="CT")
            nc.scalar.copy(CT[:], tcp[:])
            return C_bf, CT

        def s1q(b):
            s = b % NSLOT
            Qb = Q_all[:, b, :]  # [64, 256] f32
            C_bf, CT = s1c_out.pop(b)

            # s_q = rowsum(Q * wq) fused into one DVE op
            scr = sb.tile([LQ, D], bf16, tag="scr")
            s_q = sb.tile([LQ, 1], f32, tag="s_q")
            nc.vector.scalar_tensor_tensor(
                scr[:], Qb, 1.0, wq_b, op0=MUL, op1=MUL, accum_out=s_q[:]
            )

            # tq: Q'^T -> [128, 2*64]; QW = [Q'^T_k | wc_k] [128, 2, 65]
            # (the wc column of the persistent QW slot is pre-written)
            tq = ps_mm.tile([128, 128], bf16, tag="mm")
            for k in range(2):
                nc.tensor.transpose(
                    tq[:, ts(k, 64)], QP_all[:, b, ts(k, 128)], ident[0:LQ, 0:LQ]
                )
            QW = QW_all[:, s]
            nc.vector.tensor_copy(
                QW[:, :, 0:64], tq[:].rearrange("p (k j) -> p k j", k=2)
            )

            st1[b] = (C_bf, Q_bfs[:, b], s_q, QW, CT)

        def stage2(b):
            C_bf, Q_bf, s_q, QW, CT = st1.pop(b)

            # ---- M1T: s_i^T [65, 512] (row 64 = s_c^T) ----
            si_T = ps_si.tile([65, 512], f32, tag="si")
            for k in range(2):
                nc.tensor.matmul(
                    si_T[:], QW[:, k, :], CT[:, k, :], start=(k == 0), stop=(k == 1)
                )
            # E1X rows 0:64 = E1^T = exp(s_i^T + s_q) (bf16); row 64 = raw
            # s_c^T carried along so the transposes below move it for free
            E1X = sb.tile([LQ + 1, 512], bf16, tag="E1X")
            nc.scalar.activation(E1X[0:LQ, :], si_T[0:LQ, :], AF.Exp, bias=s_q[:])
            nc.scalar.copy(E1X[LQ : LQ + 1, :], si_T[LQ : LQ + 1, :])

            # E1 natural (+ s_c column) via 4 PE transposes of E1X.
            # M3 with lhsT=E1n just scales P_C rows by exp(s_q[j]), which
            # cancels in C2 = P_C[:, :256]/P_C[:, 256] — no E2 exp needed.
            e1n_ps = ps_si.tile([128, 4, LQ + 2], bf16, tag="si")
            for t in range(4):
                nc.tensor.transpose(
                    e1n_ps[:, t, 0 : LQ + 1],
                    E1X[:, ts(t, 128)],
                    ident[0 : LQ + 1, 0 : LQ + 1],
                )
            E1n = sb.tile([128, 4, LQ], bf16, tag="E1n")
            nc.vector.tensor_copy(E1n[:], e1n_ps[:, :, 0:LQ])
            # CS = exp(s_c[i]) * [C|1] row-scale: folds the s_c softmax bias
            # into the M3 rhs, since sum_i exp(si+sc)*X[i] ==
            # sum_i exp(si) * (exp(sc)*X[i]).
            exp_sc = sb.tile([128, 4, 1], f32, tag="exp_sc")
            nc.scalar.activation(exp_sc[:], e1n_ps[:, :, LQ : LQ + 1], AF.Exp)
            CS_bf = sb.tile([128, 4, D + 1], bf16, tag="CS_bf")
            cs_a, cs_b = bass.broadcast_tensor_aps(C_bf[:], exp_sc[:])
            nc.vector.tensor_mul(CS_bf[:], cs_a, cs_b)
            st2[b] = (C_bf, Q_bf, E1X, E1n, CS_bf)

        def stage3(b):
            C_bf, Q_bf, E1X, E1n, CS_bf = st2.pop(b)
            E1_T = E1X[0:LQ, :]

            # ---- M3: P_C = E1n^T @ CS -> [64, 257] (col 256 = r2) ----
            pc = ps_mm.tile([LQ, D + 1], f32, tag="mm")
            for t in range(4):
                nc.tensor.matmul(
                    pc[:], E1n[:, t, :], CS_bf[:, t, :], start=(t == 0), stop=(t == 3)
                )
            rr2 = sb.tile([LQ, 1], f32, tag="rr2")
            nc.vector.reciprocal(rr2[:], pc[:, D : D + 1])
            C2_bf = sb.tile([LQ, D], bf16, tag="C2_bf")
            nc.scalar.mul(C2_bf[:], pc[:, 0:D], rr2[:])
            st3[b] = (C_bf, Q_bf, E1_T, C2_bf)

        def stage4(b):
            C_bf, Q_bf, E1_T, C2_bf = st3.pop(b)
            OUT = stg.tile([128, 4, 3 * D], f32, tag="OUT")

            # ---- M2: P_A[t] = E1 @ [Q|1] -> [128, 257] (col 256 = r1) ----
            # A block = P_A*rr1; C*A block = (P_A*rr1)*C fused in one DVE op.
            # gpsimd cannot read PSUM, so its products read SBUF results.
            rr1 = sb.tile([128, 4, 1], f32, tag="rr1")
            for t in range(4):
                pa = ps_mm.tile([128, D + 1], f32, tag="mm")
                nc.tensor.matmul(
                    pa[:], E1_T[:, ts(t, 128)], Q_bf[:], start=True, stop=True
                )
                nc.vector.reciprocal(rr1[:, t, :], pa[:, D : D + 1])
                if t < 2:
                    nc.vector.tensor_scalar_mul(
                        OUT[:, t, 0:D], pa[:, 0:D], rr1[:, t, :]
                    )
                else:
                    nc.scalar.mul(OUT[:, t, 0:D], pa[:, 0:D], rr1[:, t, :])
                if t < 3:
                    nc.vector.scalar_tensor_tensor(
                        OUT[:, t, D : 2 * D],
                        pa[:, 0:D],
                        rr1[:, t, :],
                        C_bf[:, t, 0:D],
                        op0=MUL,
                        op1=MUL,
                    )
                else:
                    nc.gpsimd.tensor_mul(
                        OUT[:, t, D : 2 * D], OUT[:, t, 0:D], C_bf[:, t, 0:D]
                    )

            # ---- M4: P_B[t] = E1 @ C2; C*Bm = (P_B*rr1)*C fused ----
            Bm_tmp = sb.tile([128, 2, D], f32, tag="Bm_tmp")
            for th in range(2):
                pb = ps_mm.tile([128, 2, D], f32, tag="mm")
                for h in range(2):
                    t = th * 2 + h
                    nc.tensor.matmul(
                        pb[:, h, :], E1_T[:, ts(t, 128)], C2_bf[:], start=True, stop=True
                    )
                    if t < 2:
                        nc.vector.scalar_tensor_tensor(
                            OUT[:, t, 2 * D : 3 * D],
                            pb[:, h, :],
                            rr1[:, t, :],
                            C_bf[:, t, 0:D],
                            op0=MUL,
                            op1=MUL,
                        )
                    else:
                        nc.scalar.mul(Bm_tmp[:, h, :], pb[:, h, :], rr1[:, t, :])
                        nc.gpsimd.tensor_mul(
                            OUT[:, t, 2 * D : 3 * D],
                            Bm_tmp[:, h, :],
                            C_bf[:, t, 0:D],
                        )

            # ---- single 1.5MB store of [A | C*A | C*Bm] (sync ring) ----
            nc.sync.dma_start(
                out_d[b].rearrange("(p t) dd -> p t dd", t=4)[:, :, D : 4 * D],
                OUT[:],
            )

        # 4-stage software pipeline, reverse-stage emission within a step.
        # The weight-broadcast chain is emitted between batch 0's C-side and
        # Q-side work so nothing ever waits on the W loads at a queue head.
        for step in range(BL + 3):
            if step >= 3:
                stage4(step - 3)
            if 2 <= step < BL + 2:
                stage3(step - 2)
            if 1 <= step < BL + 1:
                stage2(step - 1)
            if step < BL:
                s1c_out[step] = s1c(step)
                if step == 0:
                    w_chain()
                s1q(step)

    nc.compile()
    return nc


def _get_nc():
    global _NC_CACHE
    if _NC_CACHE is None:
        _NC_CACHE = _build_nc()
    return _NC_CACHE


def _make_in_maps(contex, question, W_weight):
    contex = np.asarray(contex, dtype=np.float32)
    question = np.asarray(question, dtype=np.float32)
    W_weight = np.asarray(W_weight, dtype=np.float32)
    in_maps = []
    for c in range(NCORES):
        sl = slice(c * BL, (c + 1) * BL)
        in_maps.append(
            {
                "contex": np.ascontiguousarray(contex[sl]),
                "question": np.ascontiguousarray(question[sl]),
                "W_weight": W_weight,
            }
        )
    return in_maps


def run_spmd(contex, question, W_weight, trace=False, tmpdir=None):
    """Returns (out [64,512,1024] f32, exec_time_ns or None)."""
    from concourse.bass_utils import run_bass_kernel_spmd

    nc = _get_nc()
    in_maps = _make_in_maps(contex, question, W_weight)
    res = run_bass_kernel_spmd(
        nc, in_maps, list(range(NCORES)), trace=trace, tmpdir=tmpdir
    )
    out = np.concatenate([res.results[c]["out"] for c in range(NCORES)], axis=0)
    return out, res.exec_time_ns


def kernel(contex, question, W_weight, W_bias=None, **_unused):
    # W_bias provably has no effect on the output (it is a constant shift
    # inside both softmaxes), so it is not shipped to the device.
    out, _ = run_spmd(contex, question, W_weight, trace=False)
    return out


# revision 29
# speedup vs baseline: 1.1356x; 1.1356x over previous
"""CQAttention (BiDAF-style context-query attention) on 8 TRN2 NeuronCores.

Full shapes: contex [64, 512, 256], question [64, 64, 256],
W_weight [1, 768], W_bias [1] -> out [64, 512, 1024].

Sharding: pure data-parallel over batch, 8 batches per core.

Math notes (per batch, C=[512,256], Q=[64,256], w=[wq|wc|wi]):
  S[i,j] = sum_d C[i,d]*wi[d]*Q[j,d] + C[i].wc + Q[j].wq + b
  S1 = softmax_j(S), S2 = softmax_i(S)
  - b drops out of both softmaxes; s_c drops out of S1; s_q drops out of S2.
  - E1 = exp(s_i + s_q[j]), r1[i] = sum_j E1;  S1 = E1/r1
  - E2 = exp(s_i + s_c[i]), r2[j] = sum_i E2;  S2 = E2/r2
  - A  = S1 @ Q = (E1 @ Q)/r1
  - Bm = (S1 @ S2^T) @ C = S1 @ (S2^T @ C) = (E1 @ C2)/r1, C2 = (E2^T @ C)/r2
  r1/r2 are obtained for free as ones-columns appended to the matmul rhs.
  out = [C | A | C*A | C*Bm]

DMA design:
  - context rows are mapped i = 4p + t (partition-major): C loads move
    4KB-contiguous lines; the merged [A|C*A|C*Bm] store moves 3KB lines.
  - ALL input DMAs are issued up front (before any compute is emitted) into
    persistent tiles, so no load ever queues behind compute on its issuing
    engine.  C batch 0 rides the sync ring in parallel with Q on the
    scalar ring so batch 0 can start ASAP.
  - The C output block is stored straight from the persistent C_all input
    tile on the scalar ring (idle after the loads drain) — no copy.
  - The other three blocks are assembled in one [128, 4, 768] staging tile
    and shipped as a single 1.5MB store on the sync ring.

Emission is a 4-stage software pipeline; each "step" emits, in this order,
  S4(b-3): M2/M4 + normalization/products + store   (uses E1,C2 from b-3)
  S3(b-2): M3 + 1/r2 + C2
  S2(b-1): M1T/M1' + exps
  S1(b):   casts, Q'*wi, s_q, PE transposes of C
Reverse-stage order puts instructions whose inputs are oldest (most likely
ready) at the head of every engine queue, which keeps the in-order engines
from head-of-line blocking on same-step dependency chains.
"""

import numpy as np

B, LC, LQ, D = 64, 512, 64, 256
NCORES = 8
BL = B // NCORES  # batches per core
NSLOT = 5

_NC_CACHE = None


def _build_nc():
    import concourse.bass as bass
    import concourse.mybir as mybir
    from concourse import bacc
    from concourse import masks
    from concourse import tile
    from contextlib import ExitStack

    f32 = mybir.dt.float32
    bf16 = mybir.dt.bfloat16
    AF = mybir.ActivationFunctionType
    MUL = mybir.AluOpType.mult
    ts = bass.ts

    nc = bacc.Bacc("TRN2", target_bir_lowering=False, debug=False)
    C_d = nc.dram_tensor("contex", [BL, LC, D], f32, kind="ExternalInput")
    Q_d = nc.dram_tensor("question", [BL, LQ, D], f32, kind="ExternalInput")
    W_d = nc.dram_tensor("W_weight", [1, 3 * D], f32, kind="ExternalInput")
    out_d = nc.dram_tensor("out", [BL, LC, 4 * D], f32, kind="ExternalOutput")

    with tile.TileContext(nc) as tc, ExitStack() as ctx:
        const = ctx.enter_context(tc.tile_pool(name="const", bufs=1))
        sb = ctx.enter_context(tc.tile_pool(name="sb", bufs=NSLOT))
        stg = ctx.enter_context(tc.tile_pool(name="stg", bufs=3))
        ps_tc = ctx.enter_context(tc.tile_pool(name="ps_tc", bufs=2, space="PSUM"))
        ps_si = ctx.enter_context(tc.tile_pool(name="ps_si", bufs=2, space="PSUM"))
        ps_mm = ctx.enter_context(tc.tile_pool(name="ps_mm", bufs=4, space="PSUM"))

        # ---- all input DMAs, issued before any compute exists ----
        # sync ring: C batch 0 FIRST (it gates all of batch 0's compute),
        # then the two small weight views (small DMAs cost ~1.5us each on
        # the ring, so they go after C0 and are merged where possible)
        C_all = const.tile([128, BL, 4, D], f32, tag="C_all")
        nc.sync.dma_start(C_all[:, 0], C_d[0].rearrange("(p t) d -> p t d", t=4))
        W3 = const.tile([1, 3, D], f32, tag="W3")
        nc.sync.dma_start(W3[:], W_d.rearrange("o (k d) -> o k d", d=D))
        wc_f32 = const.tile([128, 2, 1], f32, tag="wc_f32")
        nc.sync.dma_start(
            wc_f32[:], W_d[0, D : 2 * D].rearrange("(k p o) -> p k o", p=128, o=1)
        )

        # scalar ring: Q, then the remaining C batches (4KB lines)
        Q_all = const.tile([LQ, BL, D], f32, tag="Q_all")
        nc.scalar.dma_start(Q_all[:], Q_d.rearrange("b j d -> j b d"))
        for b in range(1, BL):
            nc.scalar.dma_start(
                C_all[:, b], C_d[b].rearrange("(p t) d -> p t d", t=4)
            )

        # ---- constants ----
        ident = const.tile([128, 128], bf16, tag="ident")
        masks.make_identity(nc, ident[:])

        # persistent slotted bf16 C tiles: the ones columns are written once,
        # casts only rewrite cols 0:256 each time a slot is reused
        C_bfs = const.tile([128, NSLOT, 4, D + 1], bf16, tag="C_bfs")
        nc.gpsimd.memset(C_bfs[:, :, :, D : D + 1], 1.0)

        Q_bfs = const.tile([LQ, NSLOT, D + 1], bf16, tag="Q_bfs")
        nc.gpsimd.memset(Q_bfs[:, :, D : D + 1], 1.0)
        QW_all = const.tile([128, NSLOT, 2, 65], bf16, tag="QW_all")
        ones_row = const.tile([1, LQ], f32, tag="ones_row")
        nc.vector.memset(ones_row[:], 1.0)
        wqi = const.tile([LQ, 2, D], f32, tag="wqi")

        def w_chain():
            # broadcast wq/wi rows to 64 partitions via K=1 matmuls w/ ones
            wb_ps = ps_si.tile([LQ, 2, D], f32, tag="si")
            nc.tensor.matmul(
                wb_ps[:, 0, :], ones_row[:], W3[:, 0, :], start=True, stop=True
            )
            nc.tensor.matmul(
                wb_ps[:, 1, :], ones_row[:], W3[:, 2, :], start=True, stop=True
            )
            nc.scalar.copy(wqi[:], wb_ps[:])
            # persistent slotted QW tiles: the wc columns are written once
            for s in range(NSLOT):
                nc.vector.tensor_copy(QW_all[:, s, :, 64:65], wc_f32[:])

        wq_b = wqi[:, 0, :]  # [64, 256] rows = wq
        wi_b = wqi[:, 1, :]  # [64, 256] rows = wi

        s1c_out, st1, st2, st3 = {}, {}, {}, {}  # stage state, keyed by batch

        def s1c(b):
            s = b % NSLOT
            Cb = C_all[:, b]  # [128, 4, 256] f32
            C_bf = C_bfs[:, s]  # [128, 4, 257] bf16

            # ship output block 0 = C straight from the input tile
            # (scalar ring; it drains after the input loads finish)
            nc.scalar.dma_start(
                out_d[b].rearrange("(p t) dd -> p t dd", t=4)[:, :, 0:D], Cb
            )

            # C_bf cast in halves on two engines
            nc.vector.tensor_copy(C_bf[:, 0:2, 0:D], Cb[:, 0:2, :])
            nc.scalar.copy(C_bf[:, 2:4, 0:D], Cb[:, 2:4, :])

            # tc: C^T -> CT [128, 2, 512] (k = d-tile, free position t*128+p
            # corresponds to row i = 4p + t; consistent everywhere below).
            # Emitted before tq so the PE never waits on Q-side data first.
            tcp = ps_tc.tile([128, 2, 512], bf16, tag="tcp")
            for t in range(4):
                for k in range(2):
                    nc.tensor.transpose(
                        tcp[:, k, ts(t, 128)], C_bf[:, t, ts(k, 128)], ident[:]
                    )
            CT = sb.tile([128, 2, 512], bf16, tag="CT")
            nc.scalar.copy(CT[:], tcp[:])
            return C_bf, CT

        def s1q(b):
            s = b % NSLOT
            Qb = Q_all[:, b, :]  # [64, 256] f32
            C_bf, CT = s1c_out.pop(b)

            # per-batch Q-side prep on gpsimd (small ops; keeps DVE/ACT free)
            nc.gpsimd.tensor_copy(Q_bfs[:, s, 0:D], Qb)
            QP_bf = sb.tile([LQ, D], bf16, tag="QP_bf")
            nc.gpsimd.tensor_mul(QP_bf[:], Qb, wi_b)

            # s_q = rowsum(Q * wq) fused into one DVE op
            scr = sb.tile([LQ, D], bf16, tag="scr")
            s_q = sb.tile([LQ, 1], f32, tag="s_q")
            nc.vector.scalar_tensor_tensor(
                scr[:], Qb, 1.0, wq_b, op0=MUL, op1=MUL, accum_out=s_q[:]
            )

            # tq: Q'^T -> [128, 2*64]; QW = [Q'^T_k | wc_k] [128, 2, 65]
            # (the wc column of the persistent QW slot is pre-written)
            tq = ps_mm.tile([128, 128], bf16, tag="mm")
            for k in range(2):
                nc.tensor.transpose(
                    tq[:, ts(k, 64)], QP_bf[:, ts(k, 128)], ident[0:LQ, 0:LQ]
                )
            QW = QW_all[:, s]
            nc.vector.tensor_copy(
                QW[:, :, 0:64], tq[:].rearrange("p (k j) -> p k j", k=2)
            )

            st1[b] = (C_bf, Q_bfs[:, s], s_q, QW, CT)

        def stage2(b):
            C_bf, Q_bf, s_q, QW, CT = st1.pop(b)

            # ---- M1T: s_i^T [65, 512] (row 64 = s_c^T) ----
            si_T = ps_si.tile([65, 512], f32, tag="si")
            for k in range(2):
                nc.tensor.matmul(
                    si_T[:], QW[:, k, :], CT[:, k, :], start=(k == 0), stop=(k == 1)
                )
            # E1X rows 0:64 = E1^T = exp(s_i^T + s_q) (bf16); row 64 = raw
            # s_c^T carried along so the transposes below move it for free
            E1X = sb.tile([LQ + 1, 512], bf16, tag="E1X")
            nc.scalar.activation(E1X[0:LQ, :], si_T[0:LQ, :], AF.Exp, bias=s_q[:])
            nc.scalar.copy(E1X[LQ : LQ + 1, :], si_T[LQ : LQ + 1, :])

            # E1 natural (+ s_c column) via 4 PE transposes of E1X.
            # M3 with lhsT=E1n just scales P_C rows by exp(s_q[j]), which
            # cancels in C2 = P_C[:, :256]/P_C[:, 256] — no E2 exp needed.
            e1n_ps = ps_si.tile([128, 4, LQ + 2], bf16, tag="si")
            for t in range(4):
                nc.tensor.transpose(
                    e1n_ps[:, t, 0 : LQ + 1],
                    E1X[:, ts(t, 128)],
                    ident[0 : LQ + 1, 0 : LQ + 1],
                )
            E1n = sb.tile([128, 4, LQ], bf16, tag="E1n")
            nc.vector.tensor_copy(E1n[:], e1n_ps[:, :, 0:LQ])
            # CS = exp(s_c[i]) * [C|1] row-scale: folds the s_c softmax bias
            # into the M3 rhs, since sum_i exp(si+sc)*X[i] ==
            # sum_i exp(si) * (exp(sc)*X[i]).
            exp_sc = sb.tile([128, 4, 1], f32, tag="exp_sc")
            nc.scalar.activation(exp_sc[:], e1n_ps[:, :, LQ : LQ + 1], AF.Exp)
            CS_bf = sb.tile([128, 4, D + 1], bf16, tag="CS_bf")
            cs_a, cs_b = bass.broadcast_tensor_aps(C_bf[:], exp_sc[:])
            nc.vector.tensor_mul(CS_bf[:], cs_a, cs_b)
            st2[b] = (C_bf, Q_bf, E1X, E1n, CS_bf)

        def stage3(b):
            C_bf, Q_bf, E1X, E1n, CS_bf = st2.pop(b)
            E1_T = E1X[0:LQ, :]

            # ---- M3: P_C = E1n^T @ CS -> [64, 257] (col 256 = r2) ----
            pc = ps_mm.tile([LQ, D + 1], f32, tag="mm")
            for t in range(4):
                nc.tensor.matmul(
                    pc[:], E1n[:, t, :], CS_bf[:, t, :], start=(t == 0), stop=(t == 3)
                )
            rr2 = sb.tile([LQ, 1], f32, tag="rr2")
            nc.vector.reciprocal(rr2[:], pc[:, D : D + 1])
            C2_bf = sb.tile([LQ, D], bf16, tag="C2_bf")
            nc.scalar.mul(C2_bf[:], pc[:, 0:D], rr2[:])
            st3[b] = (C_bf, Q_bf, E1_T, C2_bf)

        def stage4(b):
            C_bf, Q_bf, E1_T, C2_bf = st3.pop(b)
            OUT = stg.tile([128, 4, 3 * D], f32, tag="OUT")

            # ---- M2: P_A[t] = E1 @ [Q|1] -> [128, 257] (col 256 = r1) ----
            # A block = P_A*rr1; C*A block = (P_A*rr1)*C fused in one DVE op.
            # gpsimd cannot read PSUM, so its products read SBUF results.
            rr1 = sb.tile([128, 4, 1], f32, tag="rr1")
            for t in range(4):
                pa = ps_mm.tile([128, D + 1], f32, tag="mm")
                nc.tensor.matmul(
                    pa[:], E1_T[:, ts(t, 128)], Q_bf[:], start=True, stop=True
                )
                nc.vector.reciprocal(rr1[:, t, :], pa[:, D : D + 1])
                if t < 2:
                    nc.vector.tensor_scalar_mul(
                        OUT[:, t, 0:D], pa[:, 0:D], rr1[:, t, :]
                    )
                else:
                    nc.scalar.mul(OUT[:, t, 0:D], pa[:, 0:D], rr1[:, t, :])
                if t < 3:
                    nc.vector.scalar_tensor_tensor(
                        OUT[:, t, D : 2 * D],
                        pa[:, 0:D],
                        rr1[:, t, :],
                        C_bf[:, t, 0:D],
                        op0=MUL,
                        op1=MUL,
                    )
                else:
                    nc.gpsimd.tensor_mul(
                        OUT[:, t, D : 2 * D], OUT[:, t, 0:D], C_bf[:, t, 0:D]
                    )

            # ---- M4: P_B[t] = E1 @ C2; C*Bm = (P_B*rr1)*C fused ----
            Bm_tmp = sb.tile([128, 2, D], f32, tag="Bm_tmp")
            for th in range(2):
                pb = ps_mm.tile([128, 2, D], f32, tag="mm")
                for h in range(2):
                    t = th * 2 + h
                    nc.tensor.matmul(
                        pb[:, h, :], E1_T[:, ts(t, 128)], C2_bf[:], start=True, stop=True
                    )
                    if t < 2:
                        nc.vector.scalar_tensor_tensor(
                            OUT[:, t, 2 * D : 3 * D],
                            pb[:, h, :],
                            rr1[:, t, :],
                            C_bf[:, t, 0:D],
                            op0=MUL,
                            op1=MUL,
                        )
                    else:
                        nc.scalar.mul(Bm_tmp[:, h, :], pb[:, h, :], rr1[:, t, :])
                        nc.gpsimd.tensor_mul(
                            OUT[:, t, 2 * D : 3 * D],
                            Bm_tmp[:, h, :],
                            C_bf[:, t, 0:D],
                        )

            # ---- single 1.5MB store of [A | C*A | C*Bm] (sync ring) ----
            nc.sync.dma_start(
                out_d[b].rearrange("(p t) dd -> p t dd", t=4)[:, :, D : 4 * D],
                OUT[:],
            )

        # 4-stage software pipeline, reverse-stage emission within a step.
        # The weight-broadcast chain is emitted between batch 0's C-side and
        # Q-side work so nothing ever waits on the W loads at a queue head.
        for step in range(BL + 3):
            if step >= 3:
                stage4(step - 3)
            if 2 <= step < BL + 2:
                stage3(step - 2)
            if 1 <= step < BL + 1:
                stage2(step - 1)
            if step < BL:
                s1c_out[step] = s1c(step)
                if step == 0:
                    w_chain()
                s1q(step)

    nc.compile()
    return nc


def _get_nc():
    global _NC_CACHE
    if _NC_CACHE is None:
        _NC_CACHE = _build_nc()
    return _NC_CACHE


def _make_in_maps(contex, question, W_weight):
    contex = np.asarray(contex, dtype=np.float32)
    question = np.asarray(question, dtype=np.float32)
    W_weight = np.asarray(W_weight, dtype=np.float32)
    in_maps = []
    for c in range(NCORES):
        sl = slice(c * BL, (c + 1) * BL)
        in_maps.append(
            {
                "contex": np.ascontiguousarray(contex[sl]),
                "question": np.ascontiguousarray(question[sl]),
                "W_weight": W_weight,
            }
        )
    return in_maps


def run_spmd(contex, question, W_weight, trace=False, tmpdir=None):
    """Returns (out [64,512,1024] f32, exec_time_ns or None)."""
    from concourse.bass_utils import run_bass_kernel_spmd

    nc = _get_nc()
    in_maps = _make_in_maps(contex, question, W_weight)
    res = run_bass_kernel_spmd(
        nc, in_maps, list(range(NCORES)), trace=trace, tmpdir=tmpdir
    )
    out = np.concatenate([res.results[c]["out"] for c in range(NCORES)], axis=0)
    return out, res.exec_time_ns


def kernel(contex, question, W_weight, W_bias=None, **_unused):
    # W_bias provably has no effect on the output (it is a constant shift
    # inside both softmaxes), so it is not shipped to the device.
    out, _ = run_spmd(contex, question, W_weight, trace=False)
    return out


# revision 31
# speedup vs baseline: 1.1602x; 1.0216x over previous
"""CQAttention (BiDAF-style context-query attention) on 8 TRN2 NeuronCores.

Full shapes: contex [64, 512, 256], question [64, 64, 256],
W_weight [1, 768], W_bias [1] -> out [64, 512, 1024].

Sharding: pure data-parallel over batch, 8 batches per core.

Math notes (per batch, C=[512,256], Q=[64,256], w=[wq|wc|wi]):
  S[i,j] = sum_d C[i,d]*wi[d]*Q[j,d] + C[i].wc + Q[j].wq + b
  S1 = softmax_j(S), S2 = softmax_i(S)
  - b drops out of both softmaxes; s_c drops out of S1; s_q drops out of S2.
  - E1 = exp(s_i + s_q[j]), r1[i] = sum_j E1;  S1 = E1/r1
  - E2 = exp(s_i + s_c[i]), r2[j] = sum_i E2;  S2 = E2/r2
  - A  = S1 @ Q = (E1 @ Q)/r1
  - Bm = (S1 @ S2^T) @ C = S1 @ (S2^T @ C) = (E1 @ C2)/r1, C2 = (E2^T @ C)/r2
  r1/r2 are obtained for free as ones-columns appended to the matmul rhs.
  out = [C | A | C*A | C*Bm]

DMA design:
  - context rows are mapped i = 4p + t (partition-major): C loads move
    4KB-contiguous lines; the merged [A|C*A|C*Bm] store moves 3KB lines.
  - ALL input DMAs are issued up front (before any compute is emitted) into
    persistent tiles, so no load ever queues behind compute on its issuing
    engine.  C batch 0 rides the sync ring in parallel with Q on the
    scalar ring so batch 0 can start ASAP.
  - The C output block is stored straight from the persistent C_all input
    tile on the scalar ring (idle after the loads drain) — no copy.
  - The other three blocks are assembled in one [128, 4, 768] staging tile
    and shipped as a single 1.5MB store on the sync ring.

Emission is a 4-stage software pipeline; each "step" emits, in this order,
  S4(b-3): M2/M4 + normalization/products + store   (uses E1,C2 from b-3)
  S3(b-2): M3 + 1/r2 + C2
  S2(b-1): M1T/M1' + exps
  S1(b):   casts, Q'*wi, s_q, PE transposes of C
Reverse-stage order puts instructions whose inputs are oldest (most likely
ready) at the head of every engine queue, which keeps the in-order engines
from head-of-line blocking on same-step dependency chains.
"""

import numpy as np

B, LC, LQ, D = 64, 512, 64, 256
NCORES = 8
BL = B // NCORES  # batches per core
NSLOT = 5

_NC_CACHE = None


def _build_nc():
    import concourse.bass as bass
    import concourse.mybir as mybir
    from concourse import bacc
    from concourse import masks
    from concourse import tile
    from contextlib import ExitStack

    f32 = mybir.dt.float32
    bf16 = mybir.dt.bfloat16
    AF = mybir.ActivationFunctionType
    MUL = mybir.AluOpType.mult
    ts = bass.ts

    nc = bacc.Bacc("TRN2", target_bir_lowering=False, debug=False)
    C_d = nc.dram_tensor("contex", [BL, LC, D], f32, kind="ExternalInput")
    Q_d = nc.dram_tensor("question", [BL, LQ, D], f32, kind="ExternalInput")
    W_d = nc.dram_tensor("W_weight", [1, 3 * D], f32, kind="ExternalInput")
    out_d = nc.dram_tensor("out", [BL, LC, 4 * D], f32, kind="ExternalOutput")

    with tile.TileContext(nc) as tc, ExitStack() as ctx:
        const = ctx.enter_context(tc.tile_pool(name="const", bufs=1))
        sb = ctx.enter_context(tc.tile_pool(name="sb", bufs=NSLOT))
        stg = ctx.enter_context(tc.tile_pool(name="stg", bufs=3))
        ps_tc = ctx.enter_context(tc.tile_pool(name="ps_tc", bufs=2, space="PSUM"))
        ps_si = ctx.enter_context(tc.tile_pool(name="ps_si", bufs=2, space="PSUM"))
        ps_mm = ctx.enter_context(tc.tile_pool(name="ps_mm", bufs=4, space="PSUM"))

        # ---- all input DMAs, issued before any compute exists ----
        # sync ring: C batch 0 FIRST (it gates all of batch 0's compute),
        # then the two small weight views (small DMAs cost ~1.5us each on
        # the ring, so they go after C0 and are merged where possible)
        C_all = const.tile([128, BL, 4, D], f32, tag="C_all")
        nc.sync.dma_start(C_all[:, 0], C_d[0].rearrange("(p t) d -> p t d", t=4))
        W3 = const.tile([1, 3, D], f32, tag="W3")
        nc.sync.dma_start(W3[:], W_d.rearrange("o (k d) -> o k d", d=D))
        wc_f32 = const.tile([128, 2, 1], f32, tag="wc_f32")
        nc.sync.dma_start(
            wc_f32[:], W_d[0, D : 2 * D].rearrange("(k p o) -> p k o", p=128, o=1)
        )

        # scalar ring: Q, then the remaining C batches (4KB lines)
        Q_all = const.tile([LQ, BL, D], f32, tag="Q_all")
        nc.scalar.dma_start(Q_all[:], Q_d.rearrange("b j d -> j b d"))
        for b in range(1, BL):
            nc.scalar.dma_start(
                C_all[:, b], C_d[b].rearrange("(p t) d -> p t d", t=4)
            )

        # ---- constants ----
        ident = const.tile([128, 128], bf16, tag="ident")
        masks.make_identity(nc, ident[:])

        # persistent slotted bf16 C tiles: the ones columns are written once,
        # casts only rewrite cols 0:256 each time a slot is reused
        C_bfs = const.tile([128, NSLOT, 4, D + 1], bf16, tag="C_bfs")
        nc.gpsimd.memset(C_bfs[:, :, :, D : D + 1], 1.0)

        Q_bfs = const.tile([LQ, NSLOT, D + 1], bf16, tag="Q_bfs")
        nc.gpsimd.memset(Q_bfs[:, :, D : D + 1], 1.0)
        QW_all = const.tile([128, NSLOT, 2, 65], bf16, tag="QW_all")
        ones_row = const.tile([1, LQ], f32, tag="ones_row")
        nc.vector.memset(ones_row[:], 1.0)
        wqi = const.tile([LQ, 2, D], f32, tag="wqi")

        def w_chain():
            # broadcast wq/wi rows to 64 partitions via K=1 matmuls w/ ones
            wb_ps = ps_si.tile([LQ, 2, D], f32, tag="si")
            nc.tensor.matmul(
                wb_ps[:, 0, :], ones_row[:], W3[:, 0, :], start=True, stop=True
            )
            nc.tensor.matmul(
                wb_ps[:, 1, :], ones_row[:], W3[:, 2, :], start=True, stop=True
            )
            nc.scalar.copy(wqi[:], wb_ps[:])
            # persistent slotted QW tiles: the wc columns are written once
            for s in range(NSLOT):
                nc.vector.tensor_copy(QW_all[:, s, :, 64:65], wc_f32[:])

        wq_b = wqi[:, 0, :]  # [64, 256] rows = wq
        wi_b = wqi[:, 1, :]  # [64, 256] rows = wi

        s1c_out, st1, st2, st3 = {}, {}, {}, {}  # stage state, keyed by batch

        def s1c(b):
            s = b % NSLOT
            Cb = C_all[:, b]  # [128, 4, 256] f32
            C_bf = C_bfs[:, s]  # [128, 4, 257] bf16

            # ship output block 0 = C straight from the input tile.  Issued
            # from the idle sync engine; interleaves with the main stores.
            nc.sync.dma_start(
                out_d[b].rearrange("(p t) dd -> p t dd", t=4)[:, :, 0:D], Cb
            )

            # C_bf cast in halves on two engines
            nc.vector.tensor_copy(C_bf[:, 0:2, 0:D], Cb[:, 0:2, :])
            nc.scalar.copy(C_bf[:, 2:4, 0:D], Cb[:, 2:4, :])

            # tc: C^T -> CT [128, 2, 512] (k = d-tile, free position t*128+p
            # corresponds to row i = 4p + t; consistent everywhere below).
            # Emitted before tq so the PE never waits on Q-side data first.
            tcp = ps_tc.tile([128, 2, 512], bf16, tag="tcp")
            for t in range(4):
                for k in range(2):
                    nc.tensor.transpose(
                        tcp[:, k, ts(t, 128)], C_bf[:, t, ts(k, 128)], ident[:]
                    )
            CT = sb.tile([128, 2, 512], bf16, tag="CT")
            nc.scalar.copy(CT[:], tcp[:])
            return C_bf, CT

        def s1q(b):
            s = b % NSLOT
            Qb = Q_all[:, b, :]  # [64, 256] f32
            C_bf, CT = s1c_out.pop(b)

            # per-batch Q-side prep on gpsimd (small ops; keeps DVE/ACT free)
            nc.gpsimd.tensor_copy(Q_bfs[:, s, 0:D], Qb)
            QP_bf = sb.tile([LQ, D], bf16, tag="QP_bf")
            nc.gpsimd.tensor_mul(QP_bf[:], Qb, wi_b)

            # s_q = rowsum(Q * wq) fused into one DVE op
            scr = sb.tile([LQ, D], bf16, tag="scr")
            s_q = sb.tile([LQ, 1], f32, tag="s_q")
            nc.vector.scalar_tensor_tensor(
                scr[:], Qb, 1.0, wq_b, op0=MUL, op1=MUL, accum_out=s_q[:]
            )

            # tq: Q'^T -> [128, 2*64]; QW = [Q'^T_k | wc_k] [128, 2, 65]
            # (the wc column of the persistent QW slot is pre-written)
            tq = ps_mm.tile([128, 128], bf16, tag="mm")
            for k in range(2):
                nc.tensor.transpose(
                    tq[:, ts(k, 64)], QP_bf[:, ts(k, 128)], ident[0:LQ, 0:LQ]
                )
            QW = QW_all[:, s]
            nc.vector.tensor_copy(
                QW[:, :, 0:64], tq[:].rearrange("p (k j) -> p k j", k=2)
            )

            st1[b] = (C_bf, Q_bfs[:, s], s_q, QW, CT)

        def stage2(b):
            C_bf, Q_bf, s_q, QW, CT = st1.pop(b)

            # ---- M1T: s_i^T [65, 512] (row 64 = s_c^T) ----
            si_T = ps_si.tile([65, 512], f32, tag="si")
            for k in range(2):
                nc.tensor.matmul(
                    si_T[:], QW[:, k, :], CT[:, k, :], start=(k == 0), stop=(k == 1)
                )
            # E1X rows 0:64 = E1^T = exp(s_i^T + s_q) (bf16); row 64 = raw
            # s_c^T carried along so the transposes below move it for free
            E1X = sb.tile([LQ + 1, 512], bf16, tag="E1X")
            nc.scalar.activation(E1X[0:LQ, :], si_T[0:LQ, :], AF.Exp, bias=s_q[:])
            nc.scalar.copy(E1X[LQ : LQ + 1, :], si_T[LQ : LQ + 1, :])

            # E1 natural (+ s_c column) via 4 PE transposes of E1X.
            # M3 with lhsT=E1n just scales P_C rows by exp(s_q[j]), which
            # cancels in C2 = P_C[:, :256]/P_C[:, 256] — no E2 exp needed.
            e1n_ps = ps_si.tile([128, 4, LQ + 2], bf16, tag="si")
            for t in range(4):
                nc.tensor.transpose(
                    e1n_ps[:, t, 0 : LQ + 1],
                    E1X[:, ts(t, 128)],
                    ident[0 : LQ + 1, 0 : LQ + 1],
                )
            # The s_c softmax bias is folded into E1n during its eviction:
            # E1n = exp(si+sq) * exp(sc[i]) row-scale, so M3's rhs can be
            # the plain [C|1] tile (sum_i exp(si+sc)*X[i] ==
            # sum_i exp(si) * exp(sc) * X[i], and the exp(sq[j]) row factor
            # of P_C cancels in C2 = P_C[:, :256]/P_C[:, 256]).
            exp_sc = sb.tile([128, 4, 1], f32, tag="exp_sc")
            nc.scalar.activation(exp_sc[:], e1n_ps[:, :, LQ : LQ + 1], AF.Exp)
            E1n = sb.tile([128, 4, LQ], bf16, tag="E1n")
            for t in range(4):
                if t < 2:
                    nc.vector.tensor_scalar_mul(
                        E1n[:, t, :], e1n_ps[:, t, 0:LQ], exp_sc[:, t, :]
                    )
                else:
                    nc.scalar.mul(E1n[:, t, :], e1n_ps[:, t, 0:LQ], exp_sc[:, t, :])
            st2[b] = (C_bf, Q_bf, E1X, E1n)

        def stage3(b):
            C_bf, Q_bf, E1X, E1n = st2.pop(b)
            E1_T = E1X[0:LQ, :]

            # ---- M3: P_C = E1n^T @ [C|1] -> [64, 257] (col 256 = r2) ----
            pc = ps_mm.tile([LQ, D + 1], f32, tag="mm")
            for t in range(4):
                nc.tensor.matmul(
                    pc[:], E1n[:, t, :], C_bf[:, t, :], start=(t == 0), stop=(t == 3)
                )
            rr2 = sb.tile([LQ, 1], f32, tag="rr2")
            nc.vector.reciprocal(rr2[:], pc[:, D : D + 1])
            C2_bf = sb.tile([LQ, D], bf16, tag="C2_bf")
            nc.scalar.mul(C2_bf[:], pc[:, 0:D], rr2[:])
            st3[b] = (C_bf, Q_bf, E1_T, C2_bf)

        def stage4(b):
            C_bf, Q_bf, E1_T, C2_bf = st3.pop(b)
            OUT = stg.tile([128, 4, 3 * D], f32, tag="OUT")

            # ---- M2: P_A[t] = E1 @ [Q|1] -> [128, 257] (col 256 = r1) ----
            # A block = P_A*rr1; C*A block = (P_A*rr1)*C fused in one DVE op.
            # gpsimd cannot read PSUM, so its products read SBUF results.
            rr1 = sb.tile([128, 4, 1], f32, tag="rr1")
            for t in range(4):
                pa = ps_mm.tile([128, D + 1], f32, tag="mm")
                nc.tensor.matmul(
                    pa[:], E1_T[:, ts(t, 128)], Q_bf[:], start=True, stop=True
                )
                nc.vector.reciprocal(rr1[:, t, :], pa[:, D : D + 1])
                if t < 2:
                    nc.vector.tensor_scalar_mul(
                        OUT[:, t, 0:D], pa[:, 0:D], rr1[:, t, :]
                    )
                else:
                    nc.scalar.mul(OUT[:, t, 0:D], pa[:, 0:D], rr1[:, t, :])
                if t < 3:
                    nc.vector.scalar_tensor_tensor(
                        OUT[:, t, D : 2 * D],
                        pa[:, 0:D],
                        rr1[:, t, :],
                        C_bf[:, t, 0:D],
                        op0=MUL,
                        op1=MUL,
                    )
                else:
                    nc.gpsimd.tensor_mul(
                        OUT[:, t, D : 2 * D], OUT[:, t, 0:D], C_bf[:, t, 0:D]
                    )

            # ---- M4: P_B[t] = E1 @ C2; C*Bm = (P_B*rr1)*C fused ----
            Bm_tmp = sb.tile([128, 2, D], f32, tag="Bm_tmp")
            for th in range(2):
                pb = ps_mm.tile([128, 2, D], f32, tag="mm")
                for h in range(2):
                    t = th * 2 + h
                    nc.tensor.matmul(
                        pb[:, h, :], E1_T[:, ts(t, 128)], C2_bf[:], start=True, stop=True
                    )
                    if t < 2:
                        nc.vector.scalar_tensor_tensor(
                            OUT[:, t, 2 * D : 3 * D],
                            pb[:, h, :],
                            rr1[:, t, :],
                            C_bf[:, t, 0:D],
                            op0=MUL,
                            op1=MUL,
                        )
                    else:
                        nc.scalar.mul(Bm_tmp[:, h, :], pb[:, h, :], rr1[:, t, :])
                        nc.gpsimd.tensor_mul(
                            OUT[:, t, 2 * D : 3 * D],
                            Bm_tmp[:, h, :],
                            C_bf[:, t, 0:D],
                        )

            # ---- single 1.5MB store of [A | C*A | C*Bm] (sync ring) ----
            nc.sync.dma_start(
                out_d[b].rearrange("(p t) dd -> p t dd", t=4)[:, :, D : 4 * D],
                OUT[:],
            )

        # 4-stage software pipeline, reverse-stage emission within a step.
        # The weight-broadcast chain is emitted between batch 0's C-side and
        # Q-side work so nothing ever waits on the W loads at a queue head.
        for step in range(BL + 3):
            if step >= 3:
                stage4(step - 3)
            if 2 <= step < BL + 2:
                stage3(step - 2)
            if 1 <= step < BL + 1:
                stage2(step - 1)
            if step < BL:
                s1c_out[step] = s1c(step)
                if step == 0:
                    w_chain()
                s1q(step)

    nc.compile()
    return nc


def _get_nc():
    global _NC_CACHE
    if _NC_CACHE is None:
        _NC_CACHE = _build_nc()
    return _NC_CACHE


def _make_in_maps(contex, question, W_weight):
    contex = np.asarray(contex, dtype=np.float32)
    question = np.asarray(question, dtype=np.float32)
    W_weight = np.asarray(W_weight, dtype=np.float32)
    in_maps = []
    for c in range(NCORES):
        sl = slice(c * BL, (c + 1) * BL)
        in_maps.append(
            {
                "contex": np.ascontiguousarray(contex[sl]),
                "question": np.ascontiguousarray(question[sl]),
                "W_weight": W_weight,
            }
        )
    return in_maps


def run_spmd(contex, question, W_weight, trace=False, tmpdir=None):
    """Returns (out [64,512,1024] f32, exec_time_ns or None)."""
    from concourse.bass_utils import run_bass_kernel_spmd

    nc = _get_nc()
    in_maps = _make_in_maps(contex, question, W_weight)
    res = run_bass_kernel_spmd(
        nc, in_maps, list(range(NCORES)), trace=trace, tmpdir=tmpdir
    )
    out = np.concatenate([res.results[c]["out"] for c in range(NCORES)], axis=0)
    return out, res.exec_time_ns


def kernel(contex, question, W_weight, W_bias=None, **_unused):
    # W_bias provably has no effect on the output (it is a constant shift
    # inside both softmaxes), so it is not shipped to the device.
    out, _ = run_spmd(contex, question, W_weight, trace=False)
    return out


# revision 35
# speedup vs baseline: 1.1805x; 1.0175x over previous
"""CQAttention (BiDAF-style context-query attention) on 8 TRN2 NeuronCores.

Full shapes: contex [64, 512, 256], question [64, 64, 256],
W_weight [1, 768], W_bias [1] -> out [64, 512, 1024].

Sharding: pure data-parallel over batch, 8 batches per core.

Math notes (per batch, C=[512,256], Q=[64,256], w=[wq|wc|wi]):
  S[i,j] = sum_d C[i,d]*wi[d]*Q[j,d] + C[i].wc + Q[j].wq + b
  S1 = softmax_j(S), S2 = softmax_i(S)
  - b drops out of both softmaxes; s_c drops out of S1; s_q drops out of S2.
  - E1 = exp(s_i + s_q[j]), r1[i] = sum_j E1;  S1 = E1/r1
  - E2 = exp(s_i + s_c[i]), r2[j] = sum_i E2;  S2 = E2/r2
  - A  = S1 @ Q = (E1 @ Q)/r1
  - Bm = (S1 @ S2^T) @ C = S1 @ (S2^T @ C) = (E1 @ C2)/r1, C2 = (E2^T @ C)/r2
  r1/r2 are obtained for free as ones-columns appended to the matmul rhs.
  out = [C | A | C*A | C*Bm]

DMA design:
  - context rows are mapped i = 4p + t (partition-major): C loads move
    4KB-contiguous lines; the merged [A|C*A|C*Bm] store moves 3KB lines.
  - ALL input DMAs are issued up front (before any compute is emitted) into
    persistent tiles, so no load ever queues behind compute on its issuing
    engine.  C batch 0 rides the sync ring in parallel with Q on the
    scalar ring so batch 0 can start ASAP.
  - The C output block is stored straight from the persistent C_all input
    tile on the scalar ring (idle after the loads drain) — no copy.
  - The other three blocks are assembled in one [128, 4, 768] staging tile
    and shipped as a single 1.5MB store on the sync ring.

Emission is a 4-stage software pipeline; each "step" emits, in this order,
  S4(b-3): M2/M4 + normalization/products + store   (uses E1,C2 from b-3)
  S3(b-2): M3 + 1/r2 + C2
  S2(b-1): M1T/M1' + exps
  S1(b):   casts, Q'*wi, s_q, PE transposes of C
Reverse-stage order puts instructions whose inputs are oldest (most likely
ready) at the head of every engine queue, which keeps the in-order engines
from head-of-line blocking on same-step dependency chains.
"""

import numpy as np

B, LC, LQ, D = 64, 512, 64, 256
NCORES = 8
BL = B // NCORES  # batches per core
NSLOT = 5

_NC_CACHE = None


def _build_nc():
    import concourse.bass as bass
    import concourse.mybir as mybir
    from concourse import bacc
    from concourse import masks
    from concourse import tile
    from contextlib import ExitStack

    f32 = mybir.dt.float32
    bf16 = mybir.dt.bfloat16
    AF = mybir.ActivationFunctionType
    MUL = mybir.AluOpType.mult
    ts = bass.ts

    nc = bacc.Bacc("TRN2", target_bir_lowering=False, debug=False)
    C_d = nc.dram_tensor("contex", [BL, LC, D], f32, kind="ExternalInput")
    Q_d = nc.dram_tensor("question", [BL, LQ, D], f32, kind="ExternalInput")
    W_d = nc.dram_tensor("W_weight", [1, 3 * D], f32, kind="ExternalInput")
    out_d = nc.dram_tensor("out", [BL, LC, 4 * D], f32, kind="ExternalOutput")

    with tile.TileContext(nc) as tc, ExitStack() as ctx:
        const = ctx.enter_context(tc.tile_pool(name="const", bufs=1))
        sb = ctx.enter_context(tc.tile_pool(name="sb", bufs=NSLOT))
        stg = ctx.enter_context(tc.tile_pool(name="stg", bufs=3))
        # PSUM: 8 banks total.  ps_x (2 banks) rotates si_T/e1n/tcp/tq whose
        # readers are all one step old; ps_pa (4 banks) holds the two M2
        # output pairs; ps_pb (2 banks) rotates the M4 pairs and pc.
        ps_x = ctx.enter_context(tc.tile_pool(name="ps_x", bufs=2, space="PSUM"))
        ps_pa = ctx.enter_context(tc.tile_pool(name="ps_pa", bufs=2, space="PSUM"))
        ps_pb = ctx.enter_context(tc.tile_pool(name="ps_pb", bufs=2, space="PSUM"))

        # ---- all input DMAs, issued before any compute exists ----
        # sync ring: C batch 0 FIRST (it gates all of batch 0's compute),
        # then the two small weight views (small DMAs cost ~1.5us each on
        # the ring, so they go after C0 and are merged where possible)
        C_all = const.tile([128, BL, 4, D], f32, tag="C_all")
        nc.sync.dma_start(C_all[:, 0], C_d[0].rearrange("(p t) d -> p t d", t=4))
        W3 = const.tile([1, 3, D], f32, tag="W3")
        nc.sync.dma_start(W3[:], W_d.rearrange("o (k d) -> o k d", d=D))
        wc_f32 = const.tile([128, 2, 1], f32, tag="wc_f32")
        nc.sync.dma_start(
            wc_f32[:], W_d[0, D : 2 * D].rearrange("(k p o) -> p k o", p=128, o=1)
        )

        # scalar ring: Q, then the remaining C batches (4KB lines)
        Q_all = const.tile([LQ, BL, D], f32, tag="Q_all")
        nc.scalar.dma_start(Q_all[:], Q_d.rearrange("b j d -> j b d"))
        for b in range(1, BL):
            nc.scalar.dma_start(
                C_all[:, b], C_d[b].rearrange("(p t) d -> p t d", t=4)
            )

        # ---- constants ----
        ident = const.tile([128, 128], bf16, tag="ident")
        masks.make_identity(nc, ident[:])

        # persistent slotted bf16 C tiles: the ones columns are written once,
        # casts only rewrite cols 0:256 each time a slot is reused
        C_bfs = const.tile([128, NSLOT, 4, D + 1], bf16, tag="C_bfs")
        nc.gpsimd.memset(C_bfs[:, :, :, D : D + 1], 1.0)

        Q_bfs = const.tile([LQ, NSLOT, D + 1], bf16, tag="Q_bfs")
        nc.gpsimd.memset(Q_bfs[:, :, D : D + 1], 1.0)
        QW_all = const.tile([128, NSLOT, 2, 65], bf16, tag="QW_all")
        ones_row = const.tile([1, LQ], f32, tag="ones_row")
        nc.vector.memset(ones_row[:], 1.0)
        wqi = const.tile([LQ, 2, D], f32, tag="wqi")

        def w_chain():
            # broadcast wq/wi rows to 64 partitions via K=1 matmuls w/ ones
            wb_ps = ps_x.tile([LQ, 2, D], f32, tag="x")
            nc.tensor.matmul(
                wb_ps[:, 0, :], ones_row[:], W3[:, 0, :], start=True, stop=True
            )
            nc.tensor.matmul(
                wb_ps[:, 1, :], ones_row[:], W3[:, 2, :], start=True, stop=True
            )
            nc.scalar.copy(wqi[:], wb_ps[:])
            # persistent slotted QW tiles: the wc columns are written once
            for s in range(NSLOT):
                nc.vector.tensor_copy(QW_all[:, s, :, 64:65], wc_f32[:])

        wq_b = wqi[:, 0, :]  # [64, 256] rows = wq
        wi_b = wqi[:, 1, :]  # [64, 256] rows = wi

        s1c_out, st1, st2, st3 = {}, {}, {}, {}  # stage state, keyed by batch

        def s1c(b):
            s = b % NSLOT
            Cb = C_all[:, b]  # [128, 4, 256] f32
            C_bf = C_bfs[:, s]  # [128, 4, 257] bf16

            # ship output block 0 = C straight from the input tile.  Issued
            # from the idle sync engine; interleaves with the main stores.
            nc.sync.dma_start(
                out_d[b].rearrange("(p t) dd -> p t dd", t=4)[:, :, 0:D], Cb
            )

            # C_bf cast in halves on two engines
            nc.vector.tensor_copy(C_bf[:, 0:2, 0:D], Cb[:, 0:2, :])
            nc.scalar.copy(C_bf[:, 2:4, 0:D], Cb[:, 2:4, :])

            # tc: C^T -> CT [128, 2, 512] (k = d-tile, free position t*128+p
            # corresponds to row i = 4p + t; consistent everywhere below).
            # Emitted before tq so the PE never waits on Q-side data first.
            tcp = ps_x.tile([128, 2, 512], bf16, tag="x")
            for t in range(4):
                for k in range(2):
                    nc.tensor.transpose(
                        tcp[:, k, ts(t, 128)], C_bf[:, t, ts(k, 128)], ident[:]
                    )
            CT = sb.tile([128, 2, 512], bf16, tag="CT")
            nc.scalar.copy(CT[:], tcp[:])
            return C_bf, CT

        def s1q(b):
            s = b % NSLOT
            Qb = Q_all[:, b, :]  # [64, 256] f32
            C_bf, CT = s1c_out.pop(b)

            # per-batch Q-side prep on gpsimd (small ops; keeps DVE/ACT free)
            nc.gpsimd.tensor_copy(Q_bfs[:, s, 0:D], Qb)
            QP_bf = sb.tile([LQ, D], bf16, tag="QP_bf")
            nc.gpsimd.tensor_mul(QP_bf[:], Qb, wi_b)

            # s_q = rowsum(Q * wq) fused into one DVE op
            scr = sb.tile([LQ, D], bf16, tag="scr")
            s_q = sb.tile([LQ, 1], f32, tag="s_q")
            nc.vector.scalar_tensor_tensor(
                scr[:], Qb, 1.0, wq_b, op0=MUL, op1=MUL, accum_out=s_q[:]
            )

            # tq: Q'^T -> [128, 2*64]; QW = [Q'^T_k | wc_k] [128, 2, 65]
            # (the wc column of the persistent QW slot is pre-written)
            tq = ps_x.tile([128, 128], bf16, tag="x")
            for k in range(2):
                nc.tensor.transpose(
                    tq[:, ts(k, 64)], QP_bf[:, ts(k, 128)], ident[0:LQ, 0:LQ]
                )
            QW = QW_all[:, s]
            nc.vector.tensor_copy(
                QW[:, :, 0:64], tq[:].rearrange("p (k j) -> p k j", k=2)
            )

            st1[b] = (C_bf, Q_bfs[:, s], s_q, QW, CT)

        def stage2(b):
            C_bf, Q_bf, s_q, QW, CT = st1.pop(b)

            # ---- M1T: s_i^T [65, 512] (row 64 = s_c^T) ----
            si_T = ps_x.tile([65, 512], f32, tag="x")
            for k in range(2):
                nc.tensor.matmul(
                    si_T[:], QW[:, k, :], CT[:, k, :], start=(k == 0), stop=(k == 1)
                )
            # E1X rows 0:64 = E1^T = exp(s_i^T + s_q) (bf16); row 64 = raw
            # s_c^T carried along so the transposes below move it for free
            E1X = sb.tile([LQ + 1, 512], bf16, tag="E1X")
            nc.scalar.activation(E1X[0:LQ, :], si_T[0:LQ, :], AF.Exp, bias=s_q[:])
            nc.scalar.copy(E1X[LQ : LQ + 1, :], si_T[LQ : LQ + 1, :])

            # E1 natural (+ s_c column) via 4 PE transposes of E1X.
            # M3 with lhsT=E1n just scales P_C rows by exp(s_q[j]), which
            # cancels in C2 = P_C[:, :256]/P_C[:, 256] — no E2 exp needed.
            e1n_ps = ps_x.tile([128, 4, LQ + 2], bf16, tag="x")
            for t in range(4):
                nc.tensor.transpose(
                    e1n_ps[:, t, 0 : LQ + 1],
                    E1X[:, ts(t, 128)],
                    ident[0 : LQ + 1, 0 : LQ + 1],
                )
            # The s_c softmax bias is folded into E1n during its eviction:
            # E1n = exp(si+sq) * exp(sc[i]) row-scale, so M3's rhs can be
            # the plain [C|1] tile (sum_i exp(si+sc)*X[i] ==
            # sum_i exp(si) * exp(sc) * X[i], and the exp(sq[j]) row factor
            # of P_C cancels in C2 = P_C[:, :256]/P_C[:, 256]).
            exp_sc = sb.tile([128, 4, 1], f32, tag="exp_sc")
            nc.scalar.activation(exp_sc[:], e1n_ps[:, :, LQ : LQ + 1], AF.Exp)
            E1n = sb.tile([128, 4, LQ], bf16, tag="E1n")
            for t in range(4):
                if t < 2:
                    nc.vector.tensor_scalar_mul(
                        E1n[:, t, :], e1n_ps[:, t, 0:LQ], exp_sc[:, t, :]
                    )
                else:
                    nc.scalar.mul(E1n[:, t, :], e1n_ps[:, t, 0:LQ], exp_sc[:, t, :])
            st2[b] = (C_bf, Q_bf, E1X, E1n)

        def stage3(b):
            C_bf, Q_bf, E1X, E1n = st2.pop(b)
            E1_T = E1X[0:LQ, :]

            # ---- M3: P_C = E1n^T @ [C|1] -> [64, 257] (col 256 = r2) ----
            pc = ps_pb.tile([LQ, D + 1], f32, tag="pb")
            for t in range(4):
                nc.tensor.matmul(
                    pc[:], E1n[:, t, :], C_bf[:, t, :], start=(t == 0), stop=(t == 3)
                )
            rr2 = sb.tile([LQ, 1], f32, tag="rr2")
            nc.vector.reciprocal(rr2[:], pc[:, D : D + 1])
            C2_bf = sb.tile([LQ, D], bf16, tag="C2_bf")
            nc.scalar.mul(C2_bf[:], pc[:, 0:D], rr2[:])
            st3[b] = (C_bf, Q_bf, E1_T, C2_bf)

        def stage4(b):
            C_bf, Q_bf, E1_T, C2_bf = st3.pop(b)
            OUT = stg.tile([128, 4, 3 * D], f32, tag="OUT")

            # ---- M2: P_A[t] = E1 @ [Q|1] -> [128, 257] (col 256 = r1) ----
            # Products are computed at t-PAIR/QUAD granularity: tensor_tensor
            # with a broadcast [128, tp, 1] scalar amortizes the ~0.3us
            # fixed cost per DVE/ACT op that per-t ops were paying.
            rr1 = sb.tile([128, 4, 1], f32, tag="rr1")
            Bm_tmp = sb.tile([128, 4, D], bf16, tag="Bm_tmp")
            pas = []
            for tp in range(2):
                pa = ps_pa.tile([128, 2, 512], f32, tag="pa")
                pas.append(pa)
                for h in range(2):
                    t = tp * 2 + h
                    nc.tensor.matmul(
                        pa[:, h, 0 : D + 1],
                        E1_T[:, ts(t, 128)],
                        Q_bf[:],
                        start=True,
                        stop=True,
                    )
                nc.vector.reciprocal(
                    rr1[:, ts(tp, 2), :], pa[:, :, D : D + 1]
                )
                # A pair = P_A * (1/r1), one broadcast TT per pair
                a_in0, a_in1 = bass.broadcast_tensor_aps(
                    pa[:, :, 0:D], rr1[:, ts(tp, 2), :]
                )
                nc.vector.tensor_mul(OUT[:, ts(tp, 2), 0:D], a_in0, a_in1)

            # C*A for all four t in one op
            nc.vector.tensor_mul(
                OUT[:, :, D : 2 * D], OUT[:, :, 0:D], C_bf[:, :, 0:D]
            )

            # ---- M4: P_B[t] = E1 @ C2; Bm pair = P_B*(1/r1) (bf16) ----
            for tp in range(2):
                pb = ps_pb.tile([128, 2, D], f32, tag="pb")
                for h in range(2):
                    t = tp * 2 + h
                    nc.tensor.matmul(
                        pb[:, h, :], E1_T[:, ts(t, 128)], C2_bf[:], start=True, stop=True
                    )
                b_in0, b_in1 = bass.broadcast_tensor_aps(
                    pb[:], rr1[:, ts(tp, 2), :]
                )
                nc.vector.tensor_mul(Bm_tmp[:, ts(tp, 2), :], b_in0, b_in1)
            # C*Bm for all four t in one gpsimd op (SBUF-only operands)
            nc.gpsimd.tensor_mul(
                OUT[:, :, 2 * D : 3 * D], Bm_tmp[:], C_bf[:, :, 0:D]
            )

            # ---- single 1.5MB store of [A | C*A | C*Bm] (sync ring) ----
            nc.sync.dma_start(
                out_d[b].rearrange("(p t) dd -> p t dd", t=4)[:, :, D : 4 * D],
                OUT[:],
            )

        # 4-stage software pipeline, reverse-stage emission within a step.
        # The weight-broadcast chain is emitted between batch 0's C-side and
        # Q-side work so nothing ever waits on the W loads at a queue head.
        for step in range(BL + 3):
            if step >= 3:
                stage4(step - 3)
            if 2 <= step < BL + 2:
                stage3(step - 2)
            if 1 <= step < BL + 1:
                stage2(step - 1)
            if step < BL:
                s1c_out[step] = s1c(step)
                if step == 0:
                    w_chain()
                s1q(step)

    nc.compile()
    return nc


def _get_nc():
    global _NC_CACHE
    if _NC_CACHE is None:
        _NC_CACHE = _build_nc()
    return _NC_CACHE


def _make_in_maps(contex, question, W_weight):
    contex = np.asarray(contex, dtype=np.float32)
    question = np.asarray(question, dtype=np.float32)
    W_weight = np.asarray(W_weight, dtype=np.float32)
    in_maps = []
    for c in range(NCORES):
        sl = slice(c * BL, (c + 1) * BL)
        in_maps.append(
            {
                "contex": np.ascontiguousarray(contex[sl]),
                "question": np.ascontiguousarray(question[sl]),
                "W_weight": W_weight,
            }
        )
    return in_maps


def run_spmd(contex, question, W_weight, trace=False, tmpdir=None):
    """Returns (out [64,512,1024] f32, exec_time_ns or None)."""
    from concourse.bass_utils import run_bass_kernel_spmd

    nc = _get_nc()
    in_maps = _make_in_maps(contex, question, W_weight)
    res = run_bass_kernel_spmd(
        nc, in_maps, list(range(NCORES)), trace=trace, tmpdir=tmpdir
    )
    out = np.concatenate([res.results[c]["out"] for c in range(NCORES)], axis=0)
    return out, res.exec_time_ns


def kernel(contex, question, W_weight, W_bias=None, **_unused):
    # W_bias provably has no effect on the output (it is a constant shift
    # inside both softmaxes), so it is not shipped to the device.
    out, _ = run_spmd(contex, question, W_weight, trace=False)
    return out


# revision 37
# speedup vs baseline: 1.2416x; 1.0518x over previous
"""CQAttention (BiDAF-style context-query attention) on 8 TRN2 NeuronCores.

Full shapes: contex [64, 512, 256], question [64, 64, 256],
W_weight [1, 768], W_bias [1] -> out [64, 512, 1024].

Sharding: pure data-parallel over batch, 8 batches per core.

Math notes (per batch, C=[512,256], Q=[64,256], w=[wq|wc|wi]):
  S[i,j] = sum_d C[i,d]*wi[d]*Q[j,d] + C[i].wc + Q[j].wq + b
  S1 = softmax_j(S), S2 = softmax_i(S)
  - b drops out of both softmaxes; s_c drops out of S1; s_q drops out of S2.
  - E1 = exp(s_i + s_q[j]), r1[i] = sum_j E1;  S1 = E1/r1
  - E2 = exp(s_i + s_c[i]), r2[j] = sum_i E2;  S2 = E2/r2
  - A  = S1 @ Q = (E1 @ Q)/r1
  - Bm = (S1 @ S2^T) @ C = S1 @ (S2^T @ C) = (E1 @ C2)/r1, C2 = (E2^T @ C)/r2
  r1/r2 are obtained for free as ones-columns appended to the matmul rhs.
  out = [C | A | C*A | C*Bm]

DMA design:
  - context rows are mapped i = 4p + t (partition-major): C loads move
    4KB-contiguous lines; the merged [A|C*A|C*Bm] store moves 3KB lines.
  - ALL input DMAs are issued up front (before any compute is emitted) into
    persistent tiles, so no load ever queues behind compute on its issuing
    engine.  C batch 0 rides the sync ring in parallel with Q on the
    scalar ring so batch 0 can start ASAP.
  - The C output block is stored straight from the persistent C_all input
    tile on the scalar ring (idle after the loads drain) — no copy.
  - The other three blocks are assembled in one [128, 4, 768] staging tile
    and shipped as a single 1.5MB store on the sync ring.

Emission is a 4-stage software pipeline; each "step" emits, in this order,
  S4(b-3): M2/M4 + normalization/products + store   (uses E1,C2 from b-3)
  S3(b-2): M3 + 1/r2 + C2
  S2(b-1): M1T/M1' + exps
  S1(b):   casts, Q'*wi, s_q, PE transposes of C
Reverse-stage order puts instructions whose inputs are oldest (most likely
ready) at the head of every engine queue, which keeps the in-order engines
from head-of-line blocking on same-step dependency chains.
"""

import numpy as np

B, LC, LQ, D = 64, 512, 64, 256
NCORES = 8
BL = B // NCORES  # batches per core
NSLOT = 5

_NC_CACHE = None


def _build_nc():
    import concourse.bass as bass
    import concourse.mybir as mybir
    from concourse import bacc
    from concourse import masks
    from concourse import tile
    from contextlib import ExitStack

    f32 = mybir.dt.float32
    bf16 = mybir.dt.bfloat16
    AF = mybir.ActivationFunctionType
    MUL = mybir.AluOpType.mult
    ts = bass.ts

    nc = bacc.Bacc("TRN2", target_bir_lowering=False, debug=False)
    C_d = nc.dram_tensor("contex", [BL, LC, D], f32, kind="ExternalInput")
    Q_d = nc.dram_tensor("question", [BL, LQ, D], f32, kind="ExternalInput")
    W_d = nc.dram_tensor("W_weight", [1, 3 * D], f32, kind="ExternalInput")
    out_d = nc.dram_tensor("out", [BL, LC, 4 * D], f32, kind="ExternalOutput")

    with tile.TileContext(nc) as tc, ExitStack() as ctx:
        const = ctx.enter_context(tc.tile_pool(name="const", bufs=1))
        sb = ctx.enter_context(tc.tile_pool(name="sb", bufs=NSLOT))
        stg = ctx.enter_context(tc.tile_pool(name="stg", bufs=3))
        # PSUM: 8 banks total.  ps_x (2 banks) rotates si_T/e1n/tcp/tq whose
        # readers are all one step old; ps_pa (4 banks) holds the two M2
        # output pairs; ps_pb (2 banks) rotates the M4 pairs and pc.
        ps_x = ctx.enter_context(tc.tile_pool(name="ps_x", bufs=2, space="PSUM"))
        ps_pa = ctx.enter_context(tc.tile_pool(name="ps_pa", bufs=2, space="PSUM"))
        ps_pb = ctx.enter_context(tc.tile_pool(name="ps_pb", bufs=2, space="PSUM"))

        # ---- all input DMAs, issued before any compute exists ----
        # sync ring: C batch 0 FIRST (it gates all of batch 0's compute),
        # then the two small weight views (small DMAs cost ~1.5us each on
        # the ring, so they go after C0 and are merged where possible)
        C_all = const.tile([128, BL, 4, D], f32, tag="C_all")
        nc.sync.dma_start(C_all[:, 0], C_d[0].rearrange("(p t) d -> p t d", t=4))
        W3 = const.tile([1, 3, D], f32, tag="W3")
        nc.sync.dma_start(W3[:], W_d.rearrange("o (k d) -> o k d", d=D))
        wc_f32 = const.tile([128, 2, 1], f32, tag="wc_f32")
        nc.sync.dma_start(
            wc_f32[:], W_d[0, D : 2 * D].rearrange("(k p o) -> p k o", p=128, o=1)
        )

        # scalar ring: Q then C batches 1-2.  The HWDGE ring FIFO is shallow:
        # a dma_start with >2 transfers outstanding BLOCKS the issuing
        # engine, so the remaining C loads are grouped and issued from
        # inside the first two steps (see s1q) once earlier loads drain.
        Q_all = const.tile([LQ, BL, D], f32, tag="Q_all")
        nc.scalar.dma_start(Q_all[:], Q_d.rearrange("b j d -> j b d"))
        nc.scalar.dma_start(
            C_all[:, 1:3], C_d[1:3].rearrange("b (p t) d -> p b t d", t=4)
        )

        def late_loads(b):
            if b == 0:
                nc.scalar.dma_start(
                    C_all[:, 3:5], C_d[3:5].rearrange("b (p t) d -> p b t d", t=4)
                )
            elif b == 1:
                nc.scalar.dma_start(
                    C_all[:, 5:8], C_d[5:8].rearrange("b (p t) d -> p b t d", t=4)
                )

        # ---- constants ----
        ident = const.tile([128, 128], bf16, tag="ident")
        masks.make_identity(nc, ident[:])

        # persistent slotted bf16 C tiles: the ones columns are written once,
        # casts only rewrite cols 0:256 each time a slot is reused
        C_bfs = const.tile([128, NSLOT, 4, D + 1], bf16, tag="C_bfs")
        nc.gpsimd.memset(C_bfs[:, :, :, D : D + 1], 1.0)

        Q_bfs = const.tile([LQ, NSLOT, D + 1], bf16, tag="Q_bfs")
        nc.gpsimd.memset(Q_bfs[:, :, D : D + 1], 1.0)
        QW_all = const.tile([128, NSLOT, 2, 65], bf16, tag="QW_all")
        ones_row = const.tile([1, LQ], f32, tag="ones_row")
        nc.vector.memset(ones_row[:], 1.0)
        wqi = const.tile([LQ, 2, D], f32, tag="wqi")

        def w_chain():
            # broadcast wq/wi rows to 64 partitions via K=1 matmuls w/ ones
            wb_ps = ps_x.tile([LQ, 2, D], f32, tag="x")
            nc.tensor.matmul(
                wb_ps[:, 0, :], ones_row[:], W3[:, 0, :], start=True, stop=True
            )
            nc.tensor.matmul(
                wb_ps[:, 1, :], ones_row[:], W3[:, 2, :], start=True, stop=True
            )
            nc.scalar.copy(wqi[:], wb_ps[:])
            # persistent slotted QW tiles: the wc columns are written once
            for s in range(NSLOT):
                nc.vector.tensor_copy(QW_all[:, s, :, 64:65], wc_f32[:])

        wq_b = wqi[:, 0, :]  # [64, 256] rows = wq
        wi_b = wqi[:, 1, :]  # [64, 256] rows = wi

        s1c_out, st1, st2, st3 = {}, {}, {}, {}  # stage state, keyed by batch

        def s1c(b):
            s = b % NSLOT
            Cb = C_all[:, b]  # [128, 4, 256] f32
            C_bf = C_bfs[:, s]  # [128, 4, 257] bf16

            # ship output block 0 = C straight from the input tile.  Issued
            # from the idle sync engine; interleaves with the main stores.
            nc.sync.dma_start(
                out_d[b].rearrange("(p t) dd -> p t dd", t=4)[:, :, 0:D], Cb
            )

            # C_bf cast in halves on two engines
            nc.vector.tensor_copy(C_bf[:, 0:2, 0:D], Cb[:, 0:2, :])
            nc.scalar.copy(C_bf[:, 2:4, 0:D], Cb[:, 2:4, :])

            # tc: C^T -> CT [128, 2, 512] (k = d-tile, free position t*128+p
            # corresponds to row i = 4p + t; consistent everywhere below).
            # Emitted before tq so the PE never waits on Q-side data first.
            tcp = ps_x.tile([128, 2, 512], bf16, tag="x")
            for t in range(4):
                for k in range(2):
                    nc.tensor.transpose(
                        tcp[:, k, ts(t, 128)], C_bf[:, t, ts(k, 128)], ident[:]
                    )
            CT = sb.tile([128, 2, 512], bf16, tag="CT")
            nc.scalar.copy(CT[:], tcp[:])
            return C_bf, CT

        def s1q(b):
            s = b % NSLOT
            Qb = Q_all[:, b, :]  # [64, 256] f32
            C_bf, CT = s1c_out.pop(b)

            # per-batch Q-side prep on gpsimd (small ops; keeps DVE/ACT free)
            nc.gpsimd.tensor_copy(Q_bfs[:, s, 0:D], Qb)
            QP_bf = sb.tile([LQ, D], bf16, tag="QP_bf")
            nc.gpsimd.tensor_mul(QP_bf[:], Qb, wi_b)

            # s_q = rowsum(Q * wq) fused into one DVE op
            scr = sb.tile([LQ, D], bf16, tag="scr")
            s_q = sb.tile([LQ, 1], f32, tag="s_q")
            nc.vector.scalar_tensor_tensor(
                scr[:], Qb, 1.0, wq_b, op0=MUL, op1=MUL, accum_out=s_q[:]
            )

            # tq: Q'^T -> [128, 2*64]; QW = [Q'^T_k | wc_k] [128, 2, 65]
            # (the wc column of the persistent QW slot is pre-written)
            tq = ps_x.tile([128, 128], bf16, tag="x")
            for k in range(2):
                nc.tensor.transpose(
                    tq[:, ts(k, 64)], QP_bf[:, ts(k, 128)], ident[0:LQ, 0:LQ]
                )
            QW = QW_all[:, s]
            nc.vector.tensor_copy(
                QW[:, :, 0:64], tq[:].rearrange("p (k j) -> p k j", k=2)
            )

            st1[b] = (C_bf, Q_bfs[:, s], s_q, QW, CT)
            late_loads(b)

        def stage2(b):
            C_bf, Q_bf, s_q, QW, CT = st1.pop(b)

            # ---- M1T: s_i^T [65, 512] (row 64 = s_c^T) ----
            si_T = ps_x.tile([65, 512], f32, tag="x")
            for k in range(2):
                nc.tensor.matmul(
                    si_T[:], QW[:, k, :], CT[:, k, :], start=(k == 0), stop=(k == 1)
                )
            # E1X rows 0:64 = E1^T = exp(s_i^T + s_q) (bf16); row 64 = raw
            # s_c^T carried along so the transposes below move it for free
            E1X = sb.tile([LQ + 1, 512], bf16, tag="E1X")
            nc.scalar.activation(E1X[0:LQ, :], si_T[0:LQ, :], AF.Exp, bias=s_q[:])
            nc.scalar.copy(E1X[LQ : LQ + 1, :], si_T[LQ : LQ + 1, :])

            # E1 natural (+ s_c column) via 4 PE transposes of E1X.
            # M3 with lhsT=E1n just scales P_C rows by exp(s_q[j]), which
            # cancels in C2 = P_C[:, :256]/P_C[:, 256] — no E2 exp needed.
            e1n_ps = ps_x.tile([128, 4, LQ + 2], bf16, tag="x")
            for t in range(4):
                nc.tensor.transpose(
                    e1n_ps[:, t, 0 : LQ + 1],
                    E1X[:, ts(t, 128)],
                    ident[0 : LQ + 1, 0 : LQ + 1],
                )
            # The s_c softmax bias is folded into E1n during its eviction:
            # E1n = exp(si+sq) * exp(sc[i]) row-scale, so M3's rhs can be
            # the plain [C|1] tile (sum_i exp(si+sc)*X[i] ==
            # sum_i exp(si) * exp(sc) * X[i], and the exp(sq[j]) row factor
            # of P_C cancels in C2 = P_C[:, :256]/P_C[:, 256]).
            exp_sc = sb.tile([128, 4, 1], f32, tag="exp_sc")
            nc.scalar.activation(exp_sc[:], e1n_ps[:, :, LQ : LQ + 1], AF.Exp)
            E1n = sb.tile([128, 4, LQ], bf16, tag="E1n")
            e_in0, e_in1 = bass.broadcast_tensor_aps(e1n_ps[:, :, 0:LQ], exp_sc[:])
            nc.vector.tensor_mul(E1n[:], e_in0, e_in1)
            st2[b] = (C_bf, Q_bf, E1X, E1n)

        def stage3(b):
            C_bf, Q_bf, E1X, E1n = st2.pop(b)
            E1_T = E1X[0:LQ, :]

            # ---- M3: P_C = E1n^T @ [C|1] -> [64, 257] (col 256 = r2) ----
            pc = ps_pb.tile([LQ, D + 1], f32, tag="pb")
            for t in range(4):
                nc.tensor.matmul(
                    pc[:], E1n[:, t, :], C_bf[:, t, :], start=(t == 0), stop=(t == 3)
                )
            rr2 = sb.tile([LQ, 1], f32, tag="rr2")
            nc.vector.reciprocal(rr2[:], pc[:, D : D + 1])
            C2_bf = sb.tile([LQ, D], bf16, tag="C2_bf")
            nc.scalar.mul(C2_bf[:], pc[:, 0:D], rr2[:])
            st3[b] = (C_bf, Q_bf, E1_T, C2_bf)

        def stage4(b):
            C_bf, Q_bf, E1_T, C2_bf = st3.pop(b)
            OUT = stg.tile([128, 4, 3 * D], f32, tag="OUT")

            # ---- M2: P_A[t] = E1 @ [Q|1] -> [128, 257] (col 256 = r1) ----
            # Products are computed at t-PAIR/QUAD granularity: tensor_tensor
            # with a broadcast [128, tp, 1] scalar amortizes the ~0.3us
            # fixed cost per DVE/ACT op that per-t ops were paying.
            rr1 = sb.tile([128, 4, 1], f32, tag="rr1")
            Bm_tmp = sb.tile([128, 4, D], bf16, tag="Bm_tmp")
            pas = []
            for tp in range(2):
                pa = ps_pa.tile([128, 2, 512], f32, tag="pa")
                pas.append(pa)
                for h in range(2):
                    t = tp * 2 + h
                    nc.tensor.matmul(
                        pa[:, h, 0 : D + 1],
                        E1_T[:, ts(t, 128)],
                        Q_bf[:],
                        start=True,
                        stop=True,
                    )
                nc.vector.reciprocal(
                    rr1[:, ts(tp, 2), :], pa[:, :, D : D + 1]
                )
                # A pair = P_A * (1/r1), one broadcast TT per pair
                a_in0, a_in1 = bass.broadcast_tensor_aps(
                    pa[:, :, 0:D], rr1[:, ts(tp, 2), :]
                )
                nc.vector.tensor_mul(OUT[:, ts(tp, 2), 0:D], a_in0, a_in1)

            # C*A for all four t in one op
            nc.vector.tensor_mul(
                OUT[:, :, D : 2 * D], OUT[:, :, 0:D], C_bf[:, :, 0:D]
            )

            # ---- M4: P_B[t] = E1 @ C2; Bm pair = P_B*(1/r1) (bf16) ----
            for tp in range(2):
                pb = ps_pb.tile([128, 2, D], f32, tag="pb")
                for h in range(2):
                    t = tp * 2 + h
                    nc.tensor.matmul(
                        pb[:, h, :], E1_T[:, ts(t, 128)], C2_bf[:], start=True, stop=True
                    )
                b_in0, b_in1 = bass.broadcast_tensor_aps(
                    pb[:], rr1[:, ts(tp, 2), :]
                )
                nc.vector.tensor_mul(Bm_tmp[:, ts(tp, 2), :], b_in0, b_in1)
            # C*Bm for all four t in one gpsimd op (SBUF-only operands)
            nc.gpsimd.tensor_mul(
                OUT[:, :, 2 * D : 3 * D], Bm_tmp[:], C_bf[:, :, 0:D]
            )

            # ---- single 1.5MB store of [A | C*A | C*Bm] (sync ring) ----
            nc.sync.dma_start(
                out_d[b].rearrange("(p t) dd -> p t dd", t=4)[:, :, D : 4 * D],
                OUT[:],
            )

        # 4-stage software pipeline, reverse-stage emission within a step.
        # The weight-broadcast chain is emitted between batch 0's C-side and
        # Q-side work so nothing ever waits on the W loads at a queue head.
        for step in range(BL + 3):
            if step >= 3:
                stage4(step - 3)
            if 2 <= step < BL + 2:
                stage3(step - 2)
            if 1 <= step < BL + 1:
                stage2(step - 1)
            if step < BL:
                s1c_out[step] = s1c(step)
                if step == 0:
                    w_chain()
                s1q(step)

    nc.compile()
    return nc


def _get_nc():
    global _NC_CACHE
    if _NC_CACHE is None:
        _NC_CACHE = _build_nc()
    return _NC_CACHE


def _make_in_maps(contex, question, W_weight):
    contex = np.asarray(contex, dtype=np.float32)
    question = np.asarray(question, dtype=np.float32)
    W_weight = np.asarray(W_weight, dtype=np.float32)
    in_maps = []
    for c in range(NCORES):
        sl = slice(c * BL, (c + 1) * BL)
        in_maps.append(
            {
                "contex": np.ascontiguousarray(contex[sl]),
                "question": np.ascontiguousarray(question[sl]),
                "W_weight": W_weight,
            }
        )
    return in_maps


def run_spmd(contex, question, W_weight, trace=False, tmpdir=None):
    """Returns (out [64,512,1024] f32, exec_time_ns or None)."""
    from concourse.bass_utils import run_bass_kernel_spmd

    nc = _get_nc()
    in_maps = _make_in_maps(contex, question, W_weight)
    res = run_bass_kernel_spmd(
        nc, in_maps, list(range(NCORES)), trace=trace, tmpdir=tmpdir
    )
    out = np.concatenate([res.results[c]["out"] for c in range(NCORES)], axis=0)
    return out, res.exec_time_ns


def kernel(contex, question, W_weight, W_bias=None, **_unused):
    # W_bias provably has no effect on the output (it is a constant shift
    # inside both softmaxes), so it is not shipped to the device.
    out, _ = run_spmd(contex, question, W_weight, trace=False)
    return out
